# revision 57
# baseline (speedup 1.0000x reference)
"""Gated multi-head self-attention on 8 Trainium2 NeuronCores.

Reference computation (per batch b of 4, N=1024 tokens, 8 heads x 64):
    q  = (x @ wq.T) * 64**-0.5            # scale folded into wqT on host
    k,v = split(x @ wkv.T)
    dots = q k^T + bias;  attn = softmax(dots)
    out  = (attn @ v) * sigmoid(x @ wg.T + bg)
    y    = out @ wo.T + bo                # bo added on host after gather

Sharding: token-sharded, zero collectives. Core c handles batch b=c//2 and
query-token half c%2 (512 query rows). Each core computes K/V for its whole
batch (the KV projection is duplicated across the 2 cores of a batch).

v2 schedule: all phases interleaved so every engine streams continuously.
The PE queue weaves dots tiles between projection groups so the Activation
engine (exp) starts ~5us in and never starves; AV matmuls slot in one
head-pair behind the exp pipeline.  Elementwise work is spread over three
engines: exp on Scalar (the only engine with activation tables), the
exp(bias) multiply on Vector (2x f16 mode), PSUM->SBUF casts on GpSimd.
Sigmoid is computed as 0.5+0.5*tanh(z/2) because Tanh lives in the same
activation table as Exp -- the Scalar engine never reloads its table
(saves ~17us of ACT_TABLE_LOAD vs the ln/exp/sigmoid mix).  The softmax
reciprocal runs on Vector (nc.vector.reciprocal), not Scalar.
Denominators come for free from 64 ones-columns appended to V inside the
AV matmul (PSUM rows 64:128 = 64 copies of the softmax denominator).
"""

import sys

if "/opt/trn_rl_repo" not in sys.path:
    sys.path.insert(0, "/opt/trn_rl_repo")

import numpy as np

import concourse.bass as bass  # noqa: F401  (AP helpers)
import concourse.mybir as mybir
import concourse.tile as tile
from concourse import bacc
from concourse.bass_utils import run_bass_kernel_spmd

F32 = mybir.dt.float32
F16 = mybir.dt.float16
AF = mybir.ActivationFunctionType
ALU = mybir.AluOpType

P = 128
HEADS = 8
DH = 64
DIM = 512
N = 1024  # tokens per batch (kv length)
NQ = 512  # query tokens per core
B = 4
N_CORES = 8
DT = DIM // P  # 4 channel tiles of 128
JT = N // P  # 8 kv-token tiles of 128
HP = HEADS // 2  # 4 head pairs
JP = JT // 2  # 4 kv-tile pairs

N_WARM = 24  # 128-free warmup matmuls to ramp the PE clock while DMAs land


def build_nc(use_tile_position=True):
    nc = bacc.Bacc(None, target_bir_lowering=False, debug=False)

    # Per-core inputs. Token order inside xbT/bT is "query half first".
    xbT_d = nc.dram_tensor("xbT", [DIM, N], F16, kind="ExternalInput")
    bT_d = nc.dram_tensor("bT", [HP, JP, P, 2, 2, NQ], F16, kind="ExternalInput")
    wqT_d = nc.dram_tensor("wqT", [DIM, DIM], F16, kind="ExternalInput")
    wkT_d = nc.dram_tensor("wkT", [DIM, DIM], F16, kind="ExternalInput")
    wvT_d = nc.dram_tensor("wvT", [DIM, DIM], F16, kind="ExternalInput")
    wgT_d = nc.dram_tensor("wgT", [DIM, DIM], F16, kind="ExternalInput")
    woT_d = nc.dram_tensor("woT", [DIM, DIM], F16, kind="ExternalInput")
    bgh_d = nc.dram_tensor("bgh", [DIM], F32, kind="ExternalInput")  # bg * 0.5
    y_d = nc.dram_tensor("y", [NQ, DIM], F16, kind="ExternalOutput")

    with tile.TileContext(nc) as tc:
        with (
            tc.tile_pool(name="const", bufs=1) as const,
            tc.tile_pool(name="work", bufs=1) as work,
            tc.tile_pool(name="attn", bufs=12) as attn_pool,
            tc.tile_pool(name="rec", bufs=2) as rec_pool,
            tc.tile_pool(name="yout", bufs=2) as yout,
            tc.tile_pool(name="ps_proj", bufs=2, space="PSUM") as ps_proj,
            tc.tile_pool(name="ps_dots", bufs=2, space="PSUM") as ps_dots,
            tc.tile_pool(name="ps_av", bufs=1, space="PSUM") as ps_av,
        ):
            # ---- constants + input DMAs (issued up front) ----------------
            warm_sb = const.tile([P, P], F16, tag="warm", name="warm")
            nc.vector.memset(warm_sb[:], 1.0)
            # preload the exp activation table before any real ACT work
            scr = const.tile([1, 2], F16, tag="scr", name="scr")
            nc.vector.memset(scr[:], 0.0)
            nc.scalar.activation(out=scr[0:1, 0:1], in_=scr[0:1, 0:1], func=AF.Exp)

            xbT = const.tile([P, DT, N], F16, tag="xbT", name="xbT")
            xbT_r = xbT_d.rearrange("(o p) m -> p o m", p=P)
            nc.sync.dma_start(xbT[:, 0:2, :], xbT_r[:, 0:2, :])
            nc.scalar.dma_start(xbT[:, 2:4, :], xbT_r[:, 2:4, :])

            # full exp(bias) stream queued behind xbT on the same queue so it
            # arrives in consumption order; 8 rotating buffers (~2 head
            # pairs ahead of the dots pipeline)
            eb_tiles = {}
            for hp in range(HP):
                for jp in range(JP):
                    t = work.tile([P, 2, 2 * NQ], F16, tag="eb", name=f"eb{hp}_{jp}", bufs=8)
                    nc.sync.dma_start(t[:], bT_d[hp, jp])
                    eb_tiles[(hp, jp)] = t

            def load_w(name, dram, eng):
                t = const.tile([P, DT, DIM], F16, tag=name, name=name)
                eng.dma_start(t[:], dram.rearrange("(o p) m -> p o m", p=P))
                return t

            # weights on the gpsimd (SWDGE) queue: Pool is idle early and this
            # keeps the Scalar/Vector engines free of DMA-dispatch work
            wkT = load_w("wkT", wkT_d, nc.gpsimd)
            wqT = load_w("wqT", wqT_d, nc.gpsimd)
            wvT = load_w("wvT", wvT_d, nc.gpsimd)
            wgT = load_w("wgT", wgT_d, nc.gpsimd)
            bgh_sb = const.tile([P, DT], F32, tag="bgh", name="bgh")
            nc.gpsimd.dma_start(bgh_sb[:], bgh_d.rearrange("(o p) -> p o", p=P))
            woT = load_w("woT", woT_d, nc.gpsimd)

            # persistent activations
            kT = [work.tile([P, N], F16, tag=f"kT{t}", name=f"kT{t}") for t in range(DT)]
            v_aug = [work.tile([P, HEADS * P], F16, tag=f"vaug{j}", name=f"vaug{j}") for j in range(JT)]
            qT = [work.tile([P, NQ], F16, tag=f"qT{t}", name=f"qT{t}") for t in range(DT)]
            thT = [work.tile([P, NQ], F16, tag=f"th{t}", name=f"th{t}") for t in range(DT)]
            sigT = [work.tile([P, NQ], F16, tag=f"sig{t}", name=f"sig{t}") for t in range(DT)]
            gatedT = [work.tile([P, NQ], F16, tag=f"gated{t}", name=f"gated{t}") for t in range(DT)]

            # ones in the FIRST 64 columns of each head block: the AV matmul
            # then puts the softmax denominators at PSUM partition base 0,
            # where the custom-DVE reciprocal can read them directly
            for jt in range(JT):
                eng = nc.vector if jt % 2 == 0 else nc.gpsimd
                eng.memset(
                    v_aug[jt].rearrange("p (h c) -> p h c", c=P)[:, :, 0:DH], 1.0
                )

            # ---- emitters ------------------------------------------------
            warm_ps = ps_proj.tile([P, 512], F32, tag="proj", name="warm_ps")

            def e_warm(n):
                for _ in range(n):
                    nc.tensor.matmul(
                        warm_ps[:, 0:P], warm_sb[:], warm_sb[:], start=True, stop=True
                    )

            def e_projK(ct, jc):
                ps = ps_proj.tile([P, 512], F32, tag="proj", name="proj")
                for kt in range(DT):
                    nc.tensor.matmul(
                        ps[:],
                        wkT[:, kt, ct * P : (ct + 1) * P],
                        xbT[:, kt, jc * 512 : (jc + 1) * 512],
                        start=(kt == 0),
                        stop=(kt == DT - 1),
                    )
                nc.vector.tensor_copy(out=kT[ct][:, jc * 512 : (jc + 1) * 512], in_=ps[:])

            def e_projQ(ct):
                ps = ps_proj.tile([P, 512], F32, tag="proj", name="proj")
                for kt in range(DT):
                    nc.tensor.matmul(
                        ps[:],
                        wqT[:, kt, ct * P : (ct + 1) * P],
                        xbT[:, kt, 0:NQ],
                        start=(kt == 0),
                        stop=(kt == DT - 1),
                    )
                nc.vector.tensor_copy(out=qT[ct][:], in_=ps[:])

            def e_projV(jt):
                ps = ps_proj.tile([P, 512], F32, tag="proj", name="proj")
                for kt in range(DT):
                    nc.tensor.matmul(
                        ps[:],
                        xbT[:, kt, jt * P : (jt + 1) * P],
                        wvT[:, kt, :],
                        start=(kt == 0),
                        stop=(kt == DT - 1),
                    )
                nc.vector.tensor_copy(
                    out=v_aug[jt].rearrange("p (h c) -> p h c", c=P)[:, :, DH:P],
                    in_=ps[:].rearrange("p (h c) -> p h c", c=DH),
                )

            def e_projG(ct):
                ps = ps_proj.tile([P, 512], F32, tag="proj", name="proj")
                for kt in range(DT):
                    nc.tensor.matmul(
                        ps[:],
                        wgT[:, kt, ct * P : (ct + 1) * P],
                        xbT[:, kt, 0:NQ],
                        start=(kt == 0),
                        stop=(kt == DT - 1),
                    )
                # sigmoid(g+bg) = 0.5 + 0.5*tanh((g+bg)/2); Tanh shares the
                # Exp activation table -> no table reload on Scalar.
                nc.scalar.activation(
                    out=thT[ct][:], in_=ps[:], func=AF.Tanh,
                    bias=bgh_sb[:, ct : ct + 1], scale=0.5,
                )
                nc.gpsimd.tensor_scalar(
                    out=sigT[ct][:], in0=thT[ct][:],
                    scalar1=0.5, scalar2=0.5, op0=ALU.mult, op1=ALU.add,
                )

            attn_cache = {}

            def e_dots(hp, pair):
                # per-jt 2-bank PSUM tiles, 2 in flight: the exp of tile k
                # overlaps the dots matmuls of tile k+1
                eb2 = eb_tiles[(hp, pair)]
                for j2 in range(2):
                    jt = pair * 2 + j2
                    dps = ps_dots.tile([P, 2 * NQ], F32, tag="dots", name="dots")
                    for s in range(2):
                        lo = s * DH
                        kw = dict(tile_position=(lo, 0)) if use_tile_position else {}
                        nc.tensor.matmul(
                            dps[:, s * NQ : (s + 1) * NQ],
                            kT[hp][lo : lo + DH, jt * P : (jt + 1) * P],
                            qT[hp][lo : lo + DH, :],
                            start=True,
                            stop=True,
                            **kw,
                        )
                    at = attn_pool.tile([P, 2 * NQ], F16, tag="attn", name="attn", bufs=4)
                    at2 = attn_pool.tile([P, 2 * NQ], F16, tag="attn2", name="attn2", bufs=16)
                    nc.scalar.activation(out=at[:], in_=dps[:], func=AF.Exp)
                    # GpSimd takes a few early bias multiplies (it cannot
                    # touch PSUM, so this SBUF-only op is its main useful
                    # work); early tiles have the most slack before their AV
                    eng = nc.gpsimd if jt < 2 else nc.vector
                    eng.tensor_tensor(at2[:], at[:], eb2[:, j2, :], ALU.mult)
                    attn_cache[(hp, jt)] = at2

            av_cur = {}

            def e_av(hp, quarter):
                if quarter == 0:
                    av_cur[hp] = ps_av.tile([P, 2 * NQ], F32, tag="av", name="av")
                av = av_cur[hp]
                for jt in range(2 * quarter, 2 * quarter + 2):
                    at = attn_cache[(hp, jt)]
                    for s in range(2):
                        h = 2 * hp + s
                        nc.tensor.matmul(
                            av[:, s * NQ : (s + 1) * NQ],
                            v_aug[jt][:, h * P : (h + 1) * P],
                            at[:, s * NQ : (s + 1) * NQ],
                            start=(jt == 0),
                            stop=(jt == JT - 1),
                        )

            def e_gating(hp):
                av = av_cur[hp]
                # rows 0:64 of av hold 64 copies of the softmax denominator
                # (ones-first v_aug layout): base partition 0, so the custom
                # DVE reciprocal can read PSUM directly.  approx_fast is ~5x
                # faster than reciprocal(); ~18-bit accuracy is plenty here.
                rec = rec_pool.tile([DH, 2 * NQ], F32, tag="rec", name="rec")
                nc.vector.reciprocal_approx_fast(out=rec[:], in_=av[0:DH, :])
                for s in range(2):
                    lo = s * DH
                    gh = gatedT[hp][lo : lo + DH, :]
                    # PSUM x SBUF: mixed spaces are exempt from the
                    # equal-base-partition rule
                    nc.vector.tensor_tensor(
                        gh, av[DH:P, s * NQ : (s + 1) * NQ],
                        rec[:, s * NQ : (s + 1) * NQ], ALU.mult
                    )
                for s in range(2):
                    lo = s * DH
                    gh = gatedT[hp][lo : lo + DH, :]
                    # SBUF x SBUF at equal base partitions -> legal on GpSimd;
                    # s=1 runs on Pool in parallel with s=0 on Vector
                    eng2 = nc.gpsimd if s == 1 else nc.vector
                    eng2.tensor_tensor(
                        gh, gh, sigT[hp][lo : lo + DH, :], ALU.mult
                    )

            y_ps = {}

            def e_y(it, cts, finish=False, pool=None):
                if it not in y_ps:
                    p = pool if pool is not None else ps_proj
                    tag = "dots" if p is ps_dots else "proj"
                    y_ps[it] = p.tile([P, 512], F32, tag=tag, name="yps")
                ps = y_ps[it]
                for ct in cts:
                    nc.tensor.matmul(
                        ps[:],
                        gatedT[ct][:, it * P : (it + 1) * P],
                        woT[:, ct, :],
                        start=(ct == 0),
                        stop=(ct == DT - 1),
                    )
                if finish:
                    ysb = yout.tile([P, 512], F16, tag="ysb", name="ysb")
                    nc.vector.tensor_copy(out=ysb[:], in_=ps[:])
                    yq = nc.sync if it % 2 == 0 else nc.scalar
                    yq.dma_start(y_d[it * P : (it + 1) * P, :], ysb[:])

            # ---- interleaved schedule -----------------------------------
            # The dots PSUM pool is a single 4-bank tile, so consecutive dots
            # pairs are separated by ~2.5us of PE filler (projections / AV
            # chunks) to cover the exp of the previous pair.  V projections
            # run early (AV of head pair 0 needs all of v_aug); K/Q/G land
            # shortly before their consumers; AV is emitted in 2-jt quarters.
            e_warm(N_WARM)
            e_projK(0, 0); e_projK(0, 1)
            e_projQ(0)
            e_dots(0, 0)
            e_projV(0); e_projV(1); e_projV(2)
            e_dots(0, 1)
            e_projV(3); e_projV(4); e_projV(5)
            e_dots(0, 2)
            e_projV(6); e_projV(7); e_projK(1, 0)
            e_dots(0, 3)
            e_projK(1, 1); e_projQ(1); e_projG(0)
            e_dots(1, 0)
            e_projK(2, 0); e_projK(2, 1); e_projQ(2)
            e_dots(1, 1)
            e_av(0, 0); e_av(0, 1); e_av(0, 2)
            e_dots(1, 2)
            e_av(0, 3); e_projK(3, 0); e_projK(3, 1)
            e_gating(0)
            e_dots(1, 3)
            e_projQ(3); e_projG(1)
            e_dots(2, 0)
            e_av(1, 0); e_av(1, 1); e_av(1, 2)
            e_dots(2, 1)
            e_av(1, 3); e_projG(2)
            e_gating(1)
            e_dots(2, 2)
            e_projG(3)
            e_dots(2, 3)
            e_av(2, 0); e_av(2, 1)
            e_dots(3, 0)
            e_av(2, 2); e_av(2, 3)
            e_gating(2)
            e_dots(3, 1)
            e_av(3, 0); e_y(0, [0, 1])
            e_dots(3, 2)
            e_av(3, 1); e_y(1, [0, 1])
            e_dots(3, 3)
            e_y(0, [2]); e_y(1, [2])
            e_av(3, 2); e_av(3, 3)
            e_y(2, [0, 1, 2], pool=ps_dots); e_y(3, [0, 1, 2], pool=ps_dots)
            e_gating(3)
            e_y(0, [3], finish=True); e_y(1, [3], finish=True)
            e_y(2, [3], finish=True); e_y(3, [3], finish=True)

    nc.compile()
    return nc


_CACHE = {}


def get_nc():
    if "nc" not in _CACHE:
        _CACHE["nc"] = build_nc()
    return _CACHE["nc"]


def make_in_maps(x, attn_bias, wq, wkv, wo, wg, bg):
    """Host-side sharding: per-core input dicts (weights shared by reference)."""
    x = np.asarray(x, np.float32)
    attn_bias = np.asarray(attn_bias, np.float32)
    scale = DH ** -0.5
    wqT = np.ascontiguousarray((np.asarray(wq, np.float32).T * scale), np.float16)
    wkvT = np.asarray(wkv, np.float32).T
    wkT = np.ascontiguousarray(wkvT[:, :DIM], np.float16)
    wvT = np.ascontiguousarray(wkvT[:, DIM:], np.float16)
    wgT = np.ascontiguousarray(np.asarray(wg, np.float32).T, np.float16)
    woT = np.ascontiguousarray(np.asarray(wo, np.float32).T, np.float16)
    bgh = np.asarray(bg, np.float32) * 0.5

    ab = np.exp(attn_bias[0])  # [H, N(i), N(j)]
    # bT[r0][h, j, i] = exp(bias)[h, i, j] with j permuted "query half first"
    bT = {}
    for r0 in (0, NQ):
        perm = np.r_[r0 : r0 + NQ, (NQ - r0) : (NQ - r0) + NQ]
        t = ab[:, r0 : r0 + NQ, :].transpose(0, 2, 1)[:, perm, :]
        t = t.reshape(4, 2, 4, 2, 128, NQ).transpose(0, 2, 4, 3, 1, 5)
        bT[r0] = np.ascontiguousarray(t, dtype=np.float16)

    in_maps = []
    for c in range(N_CORES):
        b, r0 = c // 2, (c % 2) * NQ
        perm = np.r_[r0 : r0 + NQ, (NQ - r0) : (NQ - r0) + NQ]
        xbT_c = np.ascontiguousarray(x[b][perm].T, np.float16)
        in_maps.append(
            {
                "xbT": xbT_c,
                "bT": bT[r0],
                "wqT": wqT,
                "wkT": wkT,
                "wvT": wvT,
                "wgT": wgT,
                "woT": woT,
                "bgh": bgh,
            }
        )
    return in_maps


def kernel(x, mask, attn_bias, wq, wkv, wo, bo, wg, bg, **_):
    # mask is all-ones per the problem spec; ignored.
    nc = get_nc()
    in_maps = make_in_maps(x, attn_bias, wq, wkv, wo, wg, bg)
    res = run_bass_kernel_spmd(nc, in_maps, list(range(N_CORES))).results
    y = np.empty((B, N, DIM), np.float32)
    for c in range(N_CORES):
        b, r0 = c // 2, (c % 2) * NQ
        y[b, r0 : r0 + NQ] = res[c]["y"].astype(np.float32)
    y += np.asarray(bo, np.float32)
    return y


# revision 58
# speedup vs baseline: 1.0254x; 1.0254x over previous
"""Gated multi-head self-attention on 8 Trainium2 NeuronCores.

Reference computation (per batch b of 4, N=1024 tokens, 8 heads x 64):
    q  = (x @ wq.T) * 64**-0.5            # scale folded into wqT on host
    k,v = split(x @ wkv.T)
    dots = q k^T + bias;  attn = softmax(dots)
    out  = (attn @ v) * sigmoid(x @ wg.T + bg)
    y    = out @ wo.T + bo                # bo added on host after gather

Sharding: token-sharded, zero collectives. Core c handles batch b=c//2 and
query-token half c%2 (512 query rows). Each core computes K/V for its whole
batch (the KV projection is duplicated across the 2 cores of a batch).

v2 schedule: all phases interleaved so every engine streams continuously.
The PE queue weaves dots tiles between projection groups so the Activation
engine (exp) starts ~5us in and never starves; AV matmuls slot in one
head-pair behind the exp pipeline.  Elementwise work is spread over three
engines: exp on Scalar (the only engine with activation tables), the
exp(bias) multiply on Vector (2x f16 mode), PSUM->SBUF casts on GpSimd.
Sigmoid is computed as 0.5+0.5*tanh(z/2) because Tanh lives in the same
activation table as Exp -- the Scalar engine never reloads its table
(saves ~17us of ACT_TABLE_LOAD vs the ln/exp/sigmoid mix).  The softmax
reciprocal runs on Vector (nc.vector.reciprocal), not Scalar.
Denominators come for free from 64 ones-columns appended to V inside the
AV matmul (PSUM rows 64:128 = 64 copies of the softmax denominator).
"""

import sys

if "/opt/trn_rl_repo" not in sys.path:
    sys.path.insert(0, "/opt/trn_rl_repo")

import numpy as np

import concourse.bass as bass  # noqa: F401  (AP helpers)
import concourse.mybir as mybir
import concourse.tile as tile
from concourse import bacc
from concourse.bass_utils import run_bass_kernel_spmd

F32 = mybir.dt.float32
F16 = mybir.dt.float16
AF = mybir.ActivationFunctionType
ALU = mybir.AluOpType

P = 128
HEADS = 8
DH = 64
DIM = 512
N = 1024  # tokens per batch (kv length)
NQ = 512  # query tokens per core
B = 4
N_CORES = 8
DT = DIM // P  # 4 channel tiles of 128
JT = N // P  # 8 kv-token tiles of 128
HP = HEADS // 2  # 4 head pairs
JP = JT // 2  # 4 kv-tile pairs

N_WARM = 24  # 128-free warmup matmuls to ramp the PE clock while DMAs land


def build_nc(use_tile_position=True):
    nc = bacc.Bacc(None, target_bir_lowering=False, debug=False)

    # Per-core inputs. Token order inside xbT/bT is "query half first".
    xbT_d = nc.dram_tensor("xbT", [DIM, N], F16, kind="ExternalInput")
    bT_d = nc.dram_tensor("bT", [HP, JP, P, 2, 2, NQ], F16, kind="ExternalInput")
    wqT_d = nc.dram_tensor("wqT", [DIM, DIM], F16, kind="ExternalInput")
    wkT_d = nc.dram_tensor("wkT", [DIM, DIM], F16, kind="ExternalInput")
    wvT_d = nc.dram_tensor("wvT", [DIM, DIM], F16, kind="ExternalInput")
    wgT_d = nc.dram_tensor("wgT", [DIM, DIM], F16, kind="ExternalInput")
    woT_d = nc.dram_tensor("woT", [DIM, DIM], F16, kind="ExternalInput")
    bgh_d = nc.dram_tensor("bgh", [DIM], F32, kind="ExternalInput")  # bg * 0.5
    y_d = nc.dram_tensor("y", [NQ, DIM], F16, kind="ExternalOutput")

    with tile.TileContext(nc) as tc:
        with (
            tc.tile_pool(name="const", bufs=1) as const,
            tc.tile_pool(name="work", bufs=1) as work,
            tc.tile_pool(name="attn", bufs=12) as attn_pool,
            tc.tile_pool(name="rec", bufs=2) as rec_pool,
            tc.tile_pool(name="yout", bufs=2) as yout,
            tc.tile_pool(name="ps_proj", bufs=2, space="PSUM") as ps_proj,
            tc.tile_pool(name="ps_dots", bufs=2, space="PSUM") as ps_dots,
            tc.tile_pool(name="ps_av", bufs=1, space="PSUM") as ps_av,
        ):
            # ---- constants + input DMAs (issued up front) ----------------
            warm_sb = const.tile([P, P], F16, tag="warm", name="warm")
            nc.vector.memset(warm_sb[:], 1.0)
            # preload the exp activation table before any real ACT work
            scr = const.tile([1, 2], F16, tag="scr", name="scr")
            nc.vector.memset(scr[:], 0.0)
            nc.scalar.activation(out=scr[0:1, 0:1], in_=scr[0:1, 0:1], func=AF.Exp)

            xbT = const.tile([P, DT, N], F16, tag="xbT", name="xbT")
            xbT_r = xbT_d.rearrange("(o p) m -> p o m", p=P)
            nc.sync.dma_start(xbT[:, 0:2, :], xbT_r[:, 0:2, :])
            nc.scalar.dma_start(xbT[:, 2:4, :], xbT_r[:, 2:4, :])

            # full exp(bias) stream queued behind xbT on the same queue so it
            # arrives in consumption order; 8 rotating buffers (~2 head
            # pairs ahead of the dots pipeline)
            eb_tiles = {}
            for hp in range(HP):
                for jp in range(JP):
                    t = work.tile([P, 2, 2 * NQ], F16, tag="eb", name=f"eb{hp}_{jp}", bufs=8)
                    nc.sync.dma_start(t[:], bT_d[hp, jp])
                    eb_tiles[(hp, jp)] = t

            def load_w(name, dram, eng):
                t = const.tile([P, DT, DIM], F16, tag=name, name=name)
                eng.dma_start(t[:], dram.rearrange("(o p) m -> p o m", p=P))
                return t

            # weights on the gpsimd (SWDGE) queue: Pool is idle early and this
            # keeps the Scalar/Vector engines free of DMA-dispatch work
            wkT = load_w("wkT", wkT_d, nc.gpsimd)
            wqT = load_w("wqT", wqT_d, nc.gpsimd)
            wvT = load_w("wvT", wvT_d, nc.gpsimd)
            wgT = load_w("wgT", wgT_d, nc.gpsimd)
            bgh_sb = const.tile([P, DT], F32, tag="bgh", name="bgh")
            nc.gpsimd.dma_start(bgh_sb[:], bgh_d.rearrange("(o p) -> p o", p=P))
            woT = load_w("woT", woT_d, nc.gpsimd)

            # persistent activations
            kT = [work.tile([P, N], F16, tag=f"kT{t}", name=f"kT{t}") for t in range(DT)]
            v_aug = [work.tile([P, HEADS * P], F16, tag=f"vaug{j}", name=f"vaug{j}") for j in range(JT)]
            qT = [work.tile([P, NQ], F16, tag=f"qT{t}", name=f"qT{t}") for t in range(DT)]
            thT = [work.tile([P, NQ], F16, tag=f"th{t}", name=f"th{t}") for t in range(DT)]
            sigT = [work.tile([P, NQ], F16, tag=f"sig{t}", name=f"sig{t}") for t in range(DT)]
            gatedT = [work.tile([P, NQ], F16, tag=f"gated{t}", name=f"gated{t}") for t in range(DT)]

            # ones in the FIRST 64 columns of each head block: the AV matmul
            # then puts the softmax denominators at PSUM partition base 0,
            # where the custom-DVE reciprocal can read them directly
            for jt in range(JT):
                eng = nc.vector if jt % 2 == 0 else nc.gpsimd
                eng.memset(
                    v_aug[jt].rearrange("p (h c) -> p h c", c=P)[:, :, 0:DH], 1.0
                )

            # ---- emitters ------------------------------------------------
            warm_ps = ps_proj.tile([P, 512], F32, tag="proj", name="warm_ps")

            def e_warm(n):
                for _ in range(n):
                    nc.tensor.matmul(
                        warm_ps[:, 0:P], warm_sb[:], warm_sb[:], start=True, stop=True
                    )

            def e_projK(ct, jc):
                ps = ps_proj.tile([P, 512], F32, tag="proj", name="proj")
                for kt in range(DT):
                    nc.tensor.matmul(
                        ps[:],
                        wkT[:, kt, ct * P : (ct + 1) * P],
                        xbT[:, kt, jc * 512 : (jc + 1) * 512],
                        start=(kt == 0),
                        stop=(kt == DT - 1),
                    )
                nc.vector.tensor_copy(out=kT[ct][:, jc * 512 : (jc + 1) * 512], in_=ps[:])

            def e_projQ(ct):
                ps = ps_proj.tile([P, 512], F32, tag="proj", name="proj")
                for kt in range(DT):
                    nc.tensor.matmul(
                        ps[:],
                        wqT[:, kt, ct * P : (ct + 1) * P],
                        xbT[:, kt, 0:NQ],
                        start=(kt == 0),
                        stop=(kt == DT - 1),
                    )
                nc.vector.tensor_copy(out=qT[ct][:], in_=ps[:])

            def e_projV(jt):
                ps = ps_proj.tile([P, 512], F32, tag="proj", name="proj")
                for kt in range(DT):
                    nc.tensor.matmul(
                        ps[:],
                        xbT[:, kt, jt * P : (jt + 1) * P],
                        wvT[:, kt, :],
                        start=(kt == 0),
                        stop=(kt == DT - 1),
                    )
                nc.vector.tensor_copy(
                    out=v_aug[jt].rearrange("p (h c) -> p h c", c=P)[:, :, DH:P],
                    in_=ps[:].rearrange("p (h c) -> p h c", c=DH),
                )

            def e_projG(ct):
                ps = ps_proj.tile([P, 512], F32, tag="proj", name="proj")
                for kt in range(DT):
                    nc.tensor.matmul(
                        ps[:],
                        wgT[:, kt, ct * P : (ct + 1) * P],
                        xbT[:, kt, 0:NQ],
                        start=(kt == 0),
                        stop=(kt == DT - 1),
                    )
                # sigmoid(g+bg) = 0.5 + 0.5*tanh((g+bg)/2); Tanh shares the
                # Exp activation table -> no table reload on Scalar.
                nc.scalar.activation(
                    out=thT[ct][:], in_=ps[:], func=AF.Tanh,
                    bias=bgh_sb[:, ct : ct + 1], scale=0.5,
                )
                nc.gpsimd.tensor_scalar(
                    out=sigT[ct][:], in0=thT[ct][:],
                    scalar1=0.5, scalar2=0.5, op0=ALU.mult, op1=ALU.add,
                )

            attn_cache = {}

            def e_dots(hp, pair):
                # per-jt 2-bank PSUM tiles, 2 in flight: the exp of tile k
                # overlaps the dots matmuls of tile k+1
                eb2 = eb_tiles[(hp, pair)]
                for j2 in range(2):
                    jt = pair * 2 + j2
                    dps = ps_dots.tile([P, 2 * NQ], F32, tag="dots", name="dots")
                    for s in range(2):
                        lo = s * DH
                        kw = dict(tile_position=(lo, 0)) if use_tile_position else {}
                        nc.tensor.matmul(
                            dps[:, s * NQ : (s + 1) * NQ],
                            kT[hp][lo : lo + DH, jt * P : (jt + 1) * P],
                            qT[hp][lo : lo + DH, :],
                            start=True,
                            stop=True,
                            **kw,
                        )
                    at = attn_pool.tile([P, 2 * NQ], F16, tag="attn", name="attn", bufs=4)
                    at2 = attn_pool.tile([P, 2 * NQ], F16, tag="attn2", name="attn2", bufs=16)
                    nc.scalar.activation(out=at[:], in_=dps[:], func=AF.Exp)
                    # GpSimd takes a few early bias multiplies (it cannot
                    # touch PSUM, so this SBUF-only op is its main useful
                    # work); early tiles have the most slack before their AV
                    eng = nc.gpsimd if jt < 2 and hp < 3 else nc.vector
                    eng.tensor_tensor(at2[:], at[:], eb2[:, j2, :], ALU.mult)
                    attn_cache[(hp, jt)] = at2

            av_cur = {}

            def e_av(hp, quarter):
                if quarter == 0:
                    av_cur[hp] = ps_av.tile([P, 2 * NQ], F32, tag="av", name="av")
                av = av_cur[hp]
                for jt in range(2 * quarter, 2 * quarter + 2):
                    at = attn_cache[(hp, jt)]
                    for s in range(2):
                        h = 2 * hp + s
                        nc.tensor.matmul(
                            av[:, s * NQ : (s + 1) * NQ],
                            v_aug[jt][:, h * P : (h + 1) * P],
                            at[:, s * NQ : (s + 1) * NQ],
                            start=(jt == 0),
                            stop=(jt == JT - 1),
                        )

            def e_gating(hp):
                av = av_cur[hp]
                # rows 0:64 of av hold 64 copies of the softmax denominator
                # (ones-first v_aug layout): base partition 0, so the custom
                # DVE reciprocal can read PSUM directly.  approx_fast is ~5x
                # faster than reciprocal(); ~18-bit accuracy is plenty here.
                rec = rec_pool.tile([DH, 2 * NQ], F32, tag="rec", name="rec")
                nc.vector.reciprocal_approx_fast(out=rec[:], in_=av[0:DH, :])
                for s in range(2):
                    lo = s * DH
                    gh = gatedT[hp][lo : lo + DH, :]
                    # PSUM x SBUF: mixed spaces are exempt from the
                    # equal-base-partition rule
                    nc.vector.tensor_tensor(
                        gh, av[DH:P, s * NQ : (s + 1) * NQ],
                        rec[:, s * NQ : (s + 1) * NQ], ALU.mult
                    )
                for s in range(2):
                    lo = s * DH
                    gh = gatedT[hp][lo : lo + DH, :]
                    # SBUF x SBUF at equal base partitions -> legal on GpSimd;
                    # s=1 runs on Pool in parallel with s=0 on Vector
                    eng2 = nc.gpsimd if s == 1 else nc.vector
                    eng2.tensor_tensor(
                        gh, gh, sigT[hp][lo : lo + DH, :], ALU.mult
                    )

            y_ps = {}

            def e_y(it, cts, finish=False, pool=None):
                if it not in y_ps:
                    p = pool if pool is not None else ps_proj
                    tag = "dots" if p is ps_dots else "proj"
                    y_ps[it] = p.tile([P, 512], F32, tag=tag, name="yps")
                ps = y_ps[it]
                for ct in cts:
                    nc.tensor.matmul(
                        ps[:],
                        gatedT[ct][:, it * P : (it + 1) * P],
                        woT[:, ct, :],
                        start=(ct == 0),
                        stop=(ct == DT - 1),
                    )
                if finish:
                    ysb = yout.tile([P, 512], F16, tag="ysb", name="ysb")
                    nc.vector.tensor_copy(out=ysb[:], in_=ps[:])
                    yq = nc.sync if it % 2 == 0 else nc.scalar
                    yq.dma_start(y_d[it * P : (it + 1) * P, :], ysb[:])

            # ---- interleaved schedule -----------------------------------
            # The dots PSUM pool is a single 4-bank tile, so consecutive dots
            # pairs are separated by ~2.5us of PE filler (projections / AV
            # chunks) to cover the exp of the previous pair.  V projections
            # run early (AV of head pair 0 needs all of v_aug); K/Q/G land
            # shortly before their consumers; AV is emitted in 2-jt quarters.
            e_warm(N_WARM)
            e_projK(0, 0); e_projK(0, 1)
            e_projQ(0)
            e_dots(0, 0)
            e_projV(0); e_projV(1); e_projV(2)
            e_dots(0, 1)
            e_projV(3); e_projV(4); e_projV(5)
            e_dots(0, 2)
            e_projV(6); e_projV(7); e_projK(1, 0)
            e_dots(0, 3)
            e_projK(1, 1); e_projQ(1); e_projG(0)
            e_dots(1, 0)
            e_projK(2, 0); e_projK(2, 1); e_projQ(2)
            e_dots(1, 1)
            e_av(0, 0); e_av(0, 1); e_av(0, 2)
            e_dots(1, 2)
            e_av(0, 3); e_projK(3, 0); e_projK(3, 1)
            e_gating(0)
            e_dots(1, 3)
            e_projQ(3); e_projG(1)
            e_dots(2, 0)
            e_av(1, 0); e_av(1, 1); e_av(1, 2)
            e_dots(2, 1)
            e_av(1, 3); e_projG(2)
            e_gating(1)
            e_dots(2, 2)
            e_projG(3)
            e_dots(2, 3)
            e_av(2, 0); e_av(2, 1)
            e_dots(3, 0)
            e_av(2, 2); e_av(2, 3)
            e_gating(2)
            e_dots(3, 1)
            e_av(3, 0); e_y(0, [0, 1])
            e_dots(3, 2)
            e_av(3, 1); e_y(1, [0, 1])
            e_dots(3, 3)
            e_y(0, [2]); e_y(1, [2])
            e_av(3, 2); e_av(3, 3)
            e_y(2, [0, 1, 2], pool=ps_dots); e_y(3, [0, 1, 2], pool=ps_dots)
            e_gating(3)
            e_y(0, [3], finish=True); e_y(1, [3], finish=True)
            e_y(2, [3], finish=True); e_y(3, [3], finish=True)

    nc.compile()
    return nc


_CACHE = {}


def get_nc():
    if "nc" not in _CACHE:
        _CACHE["nc"] = build_nc()
    return _CACHE["nc"]


def make_in_maps(x, attn_bias, wq, wkv, wo, wg, bg):
    """Host-side sharding: per-core input dicts (weights shared by reference)."""
    x = np.asarray(x, np.float32)
    attn_bias = np.asarray(attn_bias, np.float32)
    scale = DH ** -0.5
    wqT = np.ascontiguousarray((np.asarray(wq, np.float32).T * scale), np.float16)
    wkvT = np.asarray(wkv, np.float32).T
    wkT = np.ascontiguousarray(wkvT[:, :DIM], np.float16)
    wvT = np.ascontiguousarray(wkvT[:, DIM:], np.float16)
    wgT = np.ascontiguousarray(np.asarray(wg, np.float32).T, np.float16)
    woT = np.ascontiguousarray(np.asarray(wo, np.float32).T, np.float16)
    bgh = np.asarray(bg, np.float32) * 0.5

    ab = np.exp(attn_bias[0])  # [H, N(i), N(j)]
    # bT[r0][h, j, i] = exp(bias)[h, i, j] with j permuted "query half first"
    bT = {}
    for r0 in (0, NQ):
        perm = np.r_[r0 : r0 + NQ, (NQ - r0) : (NQ - r0) + NQ]
        t = ab[:, r0 : r0 + NQ, :].transpose(0, 2, 1)[:, perm, :]
        t = t.reshape(4, 2, 4, 2, 128, NQ).transpose(0, 2, 4, 3, 1, 5)
        bT[r0] = np.ascontiguousarray(t, dtype=np.float16)

    in_maps = []
    for c in range(N_CORES):
        b, r0 = c // 2, (c % 2) * NQ
        perm = np.r_[r0 : r0 + NQ, (NQ - r0) : (NQ - r0) + NQ]
        xbT_c = np.ascontiguousarray(x[b][perm].T, np.float16)
        in_maps.append(
            {
                "xbT": xbT_c,
                "bT": bT[r0],
                "wqT": wqT,
                "wkT": wkT,
                "wvT": wvT,
                "wgT": wgT,
                "woT": woT,
                "bgh": bgh,
            }
        )
    return in_maps


def kernel(x, mask, attn_bias, wq, wkv, wo, bo, wg, bg, **_):
    # mask is all-ones per the problem spec; ignored.
    nc = get_nc()
    in_maps = make_in_maps(x, attn_bias, wq, wkv, wo, wg, bg)
    res = run_bass_kernel_spmd(nc, in_maps, list(range(N_CORES))).results
    y = np.empty((B, N, DIM), np.float32)
    for c in range(N_CORES):
        b, r0 = c // 2, (c % 2) * NQ
        y[b, r0 : r0 + NQ] = res[c]["y"].astype(np.float32)
    y += np.asarray(bo, np.float32)
    return y
